# revision 1
# baseline (speedup 1.0000x reference)
"""Bass/Trainium2 kernel for nn_ExtractModel (soft banded edit-distance vocab matcher).

Sharding: vocab axis V=1000 split 8 x 125 across NeuronCores (partition dim = vocab).
Device computes, per core:
  - cos-similarity matmul  dot[(v,j), (i,b,s)] = ext_scaled . vocab_scaled^T   (PE, fp32)
  - diff = 0.5 - 0.5*dot                                                      (ACT)
  - banded soft edit-distance DP over 36 band cells [125 x 192] tiles         (DVE)
Band-cell DP tables are DMA'd out; host does the tiny vocab_length gather,
global min/argmin over V, scoring and argmax (negligible FLOPs).

Raw Bass (no TileContext): this toolchain's walrus rejects instructions
carrying more than one attached semaphore wait, so all cross-engine syncs are
standalone wait_ge instructions with a hand-rolled 4-semaphore protocol:
  s_in  : input DMAs done            (SP -> PE)
  s_pe  : psum accumulation groups   (PE -> ACT, +bank-reuse WAR back-edge)
  s_act : diff chunks in SBUF        (ACT -> DVE, and ACT -> PE bank release)
  s_dve : whole DP finished          (DVE -> SP stores)

The reference's second DP table (not_viable init, all-BIG) provably yields
values >= 99.9 > MATCH_THRESH everywhere, so non-viable positions always score
exactly +/-0.0 and never match; constant BIG gives identical final outputs.
Shapes hardcoded per the problem spec.
"""

import numpy as np

import concourse.bass as bass
import concourse.mybir as mybir
from concourse.bass_utils import run_bass_kernel_spmd

MSL = 10
MTL = 10
BIG = 99.9
MATCH_THRESH = 0.05
BS, L, D, V = 4, 48, 256, 1000
NCORES = 8
VC = V // NCORES          # 125 vocab words per core
M = BS * L                # 192 (b,s) positions
KC = D // 128             # 2 contraction chunks
IC = 5                    # i-pair chunks (2 i's x 192 = 384 <= 512 psum bank)
NB = 8                    # psum banks in rotation
F32 = mybir.dt.float32

# band cells of the edit-distance DP, in dependency order
BAND = [(i, j) for i in range(1, MSL + 1)
        for j in range(max(i - 2, 1), min(i + 2, MTL + 1))]
BAND_IDX = {c: n for n, c in enumerate(BAND)}
GROUPS = [(j, ic) for j in range(MTL) for ic in range(IC)]  # ACT/psum groups

_prog_cache = {}


def _build_program():
    nc = bass.Bass()
    extT = nc.dram_tensor("extT", [128, KC, MSL, M], F32, kind="ExternalInput")
    vocabT = nc.dram_tensor("vocabT", [128, KC, MTL, VC], F32, kind="ExternalInput")
    fband = nc.dram_tensor("fband", [VC, len(BAND) * M], F32, kind="ExternalOutput")

    import contextlib
    with contextlib.ExitStack() as ctx:
        ent = ctx.enter_context
        ext_t = ent(nc.sbuf_tensor("ext_t", [128, KC, MSL, M], F32))
        voc_t = ent(nc.sbuf_tensor("voc_t", [128, KC, MTL, VC], F32))
        diff = [ent(nc.sbuf_tensor(f"diff{j}", [VC, MSL, M], F32))
                for j in range(MTL)]
        fall = ent(nc.sbuf_tensor("fall", [VC, len(BAND) * M], F32))
        tmpA = ent(nc.sbuf_tensor("tmpA", [VC, M], F32))
        tmpB = ent(nc.sbuf_tensor("tmpB", [VC, M], F32))
        ps = [ent(nc.psum_tensor(f"ps{b}", [VC, 2, M], F32)) for b in range(NB)]
        s_in = ent(nc.semaphore("s_in"))
        s_pe = ent(nc.semaphore("s_pe"))
        s_act = ent(nc.semaphore("s_act"))
        s_dve = ent(nc.semaphore("s_dve"))
        s_out = ent(nc.semaphore("s_out"))

        with nc.Block() as block:

            @block.sync
            def _(sync):
                sync.dma_start(ext_t[:], extT[:]).then_inc(s_in, 16)
                sync.dma_start(voc_t[:], vocabT[:]).then_inc(s_in, 16)
                sync.wait_ge(s_dve, 1)
                ncols = len(BAND) * M
                step = ncols // 8
                for q in range(8):
                    sync.dma_start(fband[:, q * step:(q + 1) * step],
                                   fall[:, q * step:(q + 1) * step]
                                   ).then_inc(s_out, 16)
                sync.wait_ge(s_out, 128)

            @block.tensor
            def _(tensor):
                tensor.wait_ge(s_in, 32)
                for g, (j, ic) in enumerate(GROUPS):
                    if g >= NB:
                        tensor.wait_ge(s_act, g - NB + 1)  # bank g%NB released
                    for kc in range(KC):
                        mm = tensor.matmul(
                            ps[g % NB][:],
                            voc_t[:, kc, j, :],
                            ext_t[:, kc, 2 * ic:2 * ic + 2, :],
                            start=(kc == 0),
                            stop=(kc == KC - 1),
                        )
                    mm.then_inc(s_pe, 1)

            @block.scalar
            def _(scalar):
                for g, (j, ic) in enumerate(GROUPS):
                    scalar.wait_ge(s_pe, g + 1)
                    scalar.activation(
                        diff[j][:, 2 * ic:2 * ic + 2, :], ps[g % NB][:],
                        mybir.ActivationFunctionType.Copy, bias=0.5, scale=-0.5,
                    ).then_inc(s_act, 1)

            @block.vector
            def _(vector):
                Alu = mybir.AluOpType
                fmap = {}

                def pred(i, j):
                    if (i, j) in fmap:
                        return fmap[(i, j)]
                    if i == 0:
                        return float(j)
                    if j == 0:
                        return float(i)
                    return BIG  # out of band

                waited = 0
                last_cell = BAND[-1]
                for (i, j) in BAND:
                    dij = diff[j - 1][:, i - 1, :]
                    # diff chunk (j-1, (i-1)//2) is ACT group index:
                    need = (j - 1) * IC + (i - 1) // 2 + 1
                    if need > waited:
                        vector.wait_ge(s_act, need)
                        waited = need

                    sub_p = pred(i - 1, j - 1)
                    consts = [p + 1.0 for p in (pred(i - 1, j), pred(i, j - 1))
                              if isinstance(p, float) and p < BIG]
                    tens = [p for p in (pred(i - 1, j), pred(i, j - 1))
                            if not isinstance(p, float)]
                    mconst = min(consts) if consts else None

                    ops = []
                    if isinstance(sub_p, float):
                        if mconst is not None:
                            ops.append(("ts2", sub_p, mconst))
                        else:
                            ops.append(("tsadd", sub_p))
                    else:
                        ops.append(("tadd", sub_p))
                    for t in tens:
                        ops.append(("stt", t))

                    n = BAND_IDX[(i, j)]
                    fcell = fall[:, n * M:(n + 1) * M]
                    fmap[(i, j)] = fcell
                    acc = None
                    tmps = [tmpA, tmpB]
                    for k, op in enumerate(ops):
                        out = fcell if k == len(ops) - 1 else tmps[k][:]
                        if op[0] == "ts2":
                            ins = vector.tensor_scalar(
                                out, dij, op[1], op[2], Alu.add, Alu.min)
                        elif op[0] == "tsadd":
                            ins = vector.tensor_scalar_add(out, dij, op[1])
                        elif op[0] == "tadd":
                            ins = vector.tensor_add(out, op[1], dij)
                        else:
                            ins = vector.scalar_tensor_tensor(
                                out, op[1], 1.0, acc, Alu.add, Alu.min)
                        acc = out
                    if (i, j) == last_cell:
                        ins.then_inc(s_dve, 1)

    return nc


def kernel(word_repr, vocab_repr, lengths, vocab_length):
    word_repr = np.asarray(word_repr, dtype=np.float32)
    vocab_repr = np.asarray(vocab_repr, dtype=np.float32)
    lengths = np.asarray(lengths)
    vocab_length = np.asarray(vocab_length)

    # ----- host prep: window extraction + cosine pre-normalization -----
    pos = np.minimum(np.arange(L)[:, None] + np.arange(MSL)[None, :], L - 1)
    ext = word_repr[:, pos, :]                                   # [bs,L,MSL,d]
    nx = np.sqrt((ext * ext).sum(-1, dtype=np.float32)) + np.float32(1e-8)
    exts = ext / nx[..., None]
    ny = np.sqrt((vocab_repr * vocab_repr).sum(-1, dtype=np.float32)) + np.float32(1e-8)
    vocs = vocab_repr / ny[..., None]

    # extT[k, kc, i, b*L+s] ; vocabT[k, kc, j, v_local]  (d = kc*128 + k)
    extT = np.ascontiguousarray(
        exts.transpose(3, 2, 0, 1).reshape(KC, 128, MSL, M)
        .transpose(1, 0, 2, 3), dtype=np.float32)
    in_maps = []
    for c in range(NCORES):
        vs = vocs[c * VC:(c + 1) * VC]                           # [125,10,256]
        vT = np.ascontiguousarray(
            vs.transpose(2, 1, 0).reshape(KC, 128, MTL, VC)
            .transpose(1, 0, 2, 3), dtype=np.float32)
        in_maps.append({"extT": extT, "vocabT": vT})

    # ----- device: matmul + banded DP on 8 cores -----
    global _last_in_maps
    _last_in_maps = in_maps
    if "nc" not in _prog_cache:
        _prog_cache["nc"] = _build_program()
    res = run_bass_kernel_spmd(_prog_cache["nc"], in_maps, list(range(NCORES)))
    fb = np.stack([res.results[c]["fband"].reshape(VC, len(BAND), M)
                   .transpose(1, 0, 2) for c in range(NCORES)])  # [8,36,125,192]

    # ----- host finish: gather at vocab_length, min over V, score, argmax -----
    f_full = np.full((MSL + 1, MTL + 1, NCORES, VC, M), BIG, dtype=np.float32)
    for n, (i, j) in enumerate(BAND):
        f_full[i, j] = fb[:, n]
    vl = vocab_length.astype(np.int64)                           # [1000] in 1..10
    v_core = np.arange(V) // VC
    v_loc = np.arange(V) % VC
    # val2[e, v, m] = f[e+1, vl[v], v]
    val2 = f_full[np.arange(1, MSL + 1)[:, None], vl[None, :],
                  v_core[None, :], v_loc[None, :], :]            # [10,1000,192]
    value = val2.transpose(2, 0, 1).reshape(BS, L, MSL, V)

    viable = (np.arange(L)[:, None] + np.arange(MSL)[None, :])[None] \
        < lengths[:, None, None]
    value = np.where(viable[..., None], value, np.float32(BIG))

    best_value = value.min(axis=-1)
    matched_vocab = value.argmin(axis=-1)
    lens = vl[matched_vocab].astype(np.float32)
    matched = best_value < np.float32(MATCH_THRESH)
    score = lens * matched.astype(np.float32) * (np.float32(1.0) - best_value)

    sf = score.reshape(BS, -1)
    best_scores = sf.max(axis=-1)
    best_inds = sf.argmax(axis=-1).astype(np.int32)
    best_starts = best_inds // MSL
    best_ends = best_inds % MSL + best_starts
    matched_any = matched.reshape(BS, -1).any(axis=-1)
    return (best_scores.astype(np.float32), best_starts.astype(np.int32),
            best_ends.astype(np.int32), matched_any)



# revision 3
# speedup vs baseline: 2.6763x; 2.6763x over previous
"""Bass/Trainium2 kernel for nn_ExtractModel (soft banded edit-distance vocab matcher).

Sharding: vocab axis V=1000 split 8 x 125 across NeuronCores (partition dim = vocab).

v2 redesign vs the v1 baseline (123.5 us):
  - ext[b,s,i] = word_repr[b, min(s+i, L-1)] has only L=48 distinct columns per
    batch, so the cosine matmul runs over the 192 distinct (b,l) pairs plus 9
    replicated clamp columns (228 total) instead of all 1920 windowed columns:
    10x less PE work. Each DP band cell (i,j) then reads a SHIFTED slice
    dpad[:, j-1, :, (i-1)+s] of the matmul output - the window gather becomes
    an access pattern, not data movement.
  - bf16 matmul operands (4x PE throughput vs fp32; dot error ~1e-3 against a
    0.3 margin on the 0.05 match threshold).
  - fp16 DP on DVE: all cell ops are InstTensorScalarPtr/scalar_tensor_tensor
    which support the DVE 4x_2p perf mode for packed 16-bit SBUF operands.
  - Output fband in fp16, DMA'd out in 4 chunks overlapped with the DP.

Device computes, per core:
  - dot[(v), (j,b,l)] = voc_scaled . word_scaled^T  on PE (bf16 -> fp32 psum)
  - dpad = 0.5 - 0.5*dot on ACT (fp32 psum -> fp16 SBUF)
  - banded soft edit-distance DP over 36 band cells [125 x 192] on DVE (fp16)
Host does the tiny vocab_length gather, global min/argmin over V, scoring and
argmax (negligible FLOPs).

Raw Bass (no TileContext): this toolchain's walrus rejects instructions
carrying more than one attached semaphore wait, so all cross-engine syncs are
standalone wait_ge instructions:
  s_in  : word + vocab(j 0..4) input DMAs done   (SP -> PE)
  s_in2 : vocab(j 5..9) input DMA done           (SP -> PE)
  s_pe  : psum accumulation groups               (PE -> ACT, +bank-reuse WAR back-edge)
  s_act : dpad j-slices in SBUF                  (ACT -> DVE, and ACT -> PE bank release)
  s_dve : band cells finished                    (DVE -> SP stores, 9-cell chunks)
  s_out : output DMAs done

The reference's second DP table (not_viable init, all-BIG) provably yields
values >= 99.9 > MATCH_THRESH everywhere, so non-viable positions always score
exactly +/-0.0 and never match; constant BIG gives identical final outputs.
Shapes hardcoded per the problem spec.
"""

import numpy as np
import ml_dtypes

import concourse.bass as bass
import concourse.mybir as mybir
from concourse.bass_utils import run_bass_kernel_spmd

MSL = 10
MTL = 10
BIG = 99.9
MATCH_THRESH = 0.05
BS, L, D, V = 4, 48, 256, 1000
NCORES = 8
VC = V // NCORES          # 125 vocab words per core
M = BS * L                # 192 (b,s) positions
LP = L + MSL - 1          # 57 padded l columns (48 real + 9 clamp copies)
MP = BS * LP              # 228 matmul moving columns
KC = D // 128             # 2 contraction chunks
NB = 8                    # psum banks in rotation
F32 = mybir.dt.float32
F16 = mybir.dt.float16
BF16 = mybir.dt.bfloat16

# band cells of the edit-distance DP, in dependency order
BAND = [(i, j) for i in range(1, MSL + 1)
        for j in range(max(i - 2, 1), min(i + 2, MTL + 1))]
BAND_IDX = {c: n for n, c in enumerate(BAND)}
NCELL = len(BAND)         # 36
CHUNK = 9                 # band cells per output DMA chunk

_prog_cache = {}


def _build_program():
    nc = bass.Bass()
    wordT = nc.dram_tensor("wordT", [128, KC, BS, LP], BF16, kind="ExternalInput")
    vocabT = nc.dram_tensor("vocabT", [128, KC, MTL, VC], BF16, kind="ExternalInput")
    fband = nc.dram_tensor("fband", [VC, NCELL, BS, L], F16, kind="ExternalOutput")

    import contextlib
    with contextlib.ExitStack() as ctx:
        ent = ctx.enter_context
        word_s = ent(nc.sbuf_tensor("word_s", [128, KC, BS, LP], BF16))
        voc_s = ent(nc.sbuf_tensor("voc_s", [128, KC, MTL, VC], BF16))
        dpad = ent(nc.sbuf_tensor("dpad", [VC, MTL, BS, LP], F16))
        fall = ent(nc.sbuf_tensor("fall", [VC, NCELL, BS, L], F16))
        tmpA = ent(nc.sbuf_tensor("tmpA", [VC, BS, L], F16))
        tmpB = ent(nc.sbuf_tensor("tmpB", [VC, BS, L], F16))
        ps = [ent(nc.psum_tensor(f"ps{b}", [VC, BS, LP], F32)) for b in range(NB)]
        s_in = ent(nc.semaphore("s_in"))
        s_in2 = ent(nc.semaphore("s_in2"))
        s_pe = ent(nc.semaphore("s_pe"))
        s_act = ent(nc.semaphore("s_act"))
        s_dve = ent(nc.semaphore("s_dve"))
        s_out = ent(nc.semaphore("s_out"))

        with nc.Block() as block:

            @block.sync
            def _(sync):
                sync.dma_start(word_s[:], wordT[:]).then_inc(s_in, 16)
                sync.dma_start(voc_s[:, :, 0:5, :], vocabT[:, :, 0:5, :]
                               ).then_inc(s_in, 16)
                sync.dma_start(voc_s[:, :, 5:10, :], vocabT[:, :, 5:10, :]
                               ).then_inc(s_in2, 16)
                nq = NCELL // CHUNK
                for q in range(nq):
                    sync.wait_ge(s_dve, CHUNK * (q + 1))
                    sync.dma_start(fband[:, q * CHUNK:(q + 1) * CHUNK],
                                   fall[:, q * CHUNK:(q + 1) * CHUNK]
                                   ).then_inc(s_out, 16)
                sync.wait_ge(s_out, 16 * nq)

            @block.tensor
            def _(tensor):
                tensor.wait_ge(s_in, 32)
                for j in range(MTL):
                    if j == 5:
                        tensor.wait_ge(s_in2, 16)
                    if j >= NB:
                        tensor.wait_ge(s_act, j - NB + 1)  # bank j%NB released
                    for kc in range(KC):
                        mm = tensor.matmul(
                            ps[j % NB][:],
                            voc_s[:, kc, j, :],
                            word_s[:, kc, :, :],
                            start=(kc == 0),
                            stop=(kc == KC - 1),
                        )
                    mm.then_inc(s_pe, 1)

            @block.scalar
            def _(scalar):
                for j in range(MTL):
                    scalar.wait_ge(s_pe, j + 1)
                    scalar.activation(
                        dpad[:, j], ps[j % NB][:],
                        mybir.ActivationFunctionType.Copy, bias=0.5, scale=-0.5,
                    ).then_inc(s_act, 1)

            @block.vector
            def _(vector):
                Alu = mybir.AluOpType
                fmap = {}

                def pred(i, j):
                    if (i, j) in fmap:
                        return fmap[(i, j)]
                    if i == 0:
                        return float(j)
                    if j == 0:
                        return float(i)
                    return BIG  # out of band

                waited = 0
                for (i, j) in BAND:
                    dij = dpad[:, j - 1, :, i - 1:i - 1 + L]
                    if j > waited:
                        vector.wait_ge(s_act, j)
                        waited = j

                    sub_p = pred(i - 1, j - 1)
                    consts = [p + 1.0 for p in (pred(i - 1, j), pred(i, j - 1))
                              if isinstance(p, float) and p < BIG]
                    tens = [p for p in (pred(i - 1, j), pred(i, j - 1))
                            if not isinstance(p, float)]
                    mconst = min(consts) if consts else None

                    n = BAND_IDX[(i, j)]
                    fcell = fall[:, n]
                    fmap[(i, j)] = fcell
                    tmps = [tmpA[:], tmpB[:]]
                    nops = 1 + len(tens)
                    k = 0

                    def out_of(k):
                        return fcell if k == nops - 1 else tmps[k]

                    # combine substitution pred (and const ins/del mins)
                    if isinstance(sub_p, float):
                        if mconst is not None:
                            ins = vector.tensor_scalar(
                                out_of(k), dij, sub_p, mconst, Alu.add, Alu.min)
                        else:
                            ins = vector.tensor_scalar_add(out_of(k), dij, sub_p)
                    else:
                        # sub + dij via TensorScalarPtr (4x fp16 mode)
                        ins = vector.scalar_tensor_tensor(
                            out_of(k), sub_p, 0.0, dij, Alu.add, Alu.add)
                    acc = out_of(k)
                    k += 1
                    # chain tensor ins/del preds: acc = min(t + 1, acc)
                    for t in tens:
                        ins = vector.scalar_tensor_tensor(
                            out_of(k), t, 1.0, acc, Alu.add, Alu.min)
                        acc = out_of(k)
                        k += 1
                    ins.then_inc(s_dve, 1)

    return nc


def kernel(word_repr, vocab_repr, lengths, vocab_length):
    word_repr = np.asarray(word_repr, dtype=np.float32)
    vocab_repr = np.asarray(vocab_repr, dtype=np.float32)
    lengths = np.asarray(lengths)
    vocab_length = np.asarray(vocab_length)

    # ----- host prep: cosine pre-normalization + clamp-column replication -----
    nx = np.sqrt((word_repr * word_repr).sum(-1, dtype=np.float32)) + np.float32(1e-8)
    wordn = word_repr / nx[..., None]                            # [bs,L,d]
    ny = np.sqrt((vocab_repr * vocab_repr).sum(-1, dtype=np.float32)) + np.float32(1e-8)
    vocn = vocab_repr / ny[..., None]                            # [V,MTL,d]

    # wpad[b, l', d]: l' 0..47 real, 48..56 copies of column 47 (window clamp)
    wpad = np.concatenate(
        [wordn, np.repeat(wordn[:, L - 1:L, :], MSL - 1, axis=1)], axis=1)
    # wordT[k, kc, b, l']  (d = kc*128 + k)
    wordT = np.ascontiguousarray(
        wpad.transpose(2, 0, 1).reshape(KC, 128, BS, LP)
        .transpose(1, 0, 2, 3)).astype(ml_dtypes.bfloat16)
    in_maps = []
    for c in range(NCORES):
        vs = vocn[c * VC:(c + 1) * VC]                           # [125,10,256]
        vT = np.ascontiguousarray(
            vs.transpose(2, 1, 0).reshape(KC, 128, MTL, VC)
            .transpose(1, 0, 2, 3)).astype(ml_dtypes.bfloat16)
        in_maps.append({"wordT": wordT, "vocabT": vT})

    # ----- device: matmul + banded DP on 8 cores -----
    global _last_in_maps
    _last_in_maps = in_maps
    if "nc" not in _prog_cache:
        _prog_cache["nc"] = _build_program()
    res = run_bass_kernel_spmd(_prog_cache["nc"], in_maps, list(range(NCORES)))
    fb = np.stack([res.results[c]["fband"].reshape(VC, NCELL, M)
                   .transpose(1, 0, 2) for c in range(NCORES)]
                  ).astype(np.float32)                           # [8,36,125,192]

    # ----- host finish: gather at vocab_length, min over V, score, argmax -----
    f_full = np.full((MSL + 1, MTL + 1, NCORES, VC, M), BIG, dtype=np.float32)
    for n, (i, j) in enumerate(BAND):
        f_full[i, j] = fb[:, n]
    vl = vocab_length.astype(np.int64)                           # [1000] in 1..10
    v_core = np.arange(V) // VC
    v_loc = np.arange(V) % VC
    # val2[e, v, m] = f[e+1, vl[v], v]
    val2 = f_full[np.arange(1, MSL + 1)[:, None], vl[None, :],
                  v_core[None, :], v_loc[None, :], :]            # [10,1000,192]
    value = val2.transpose(2, 0, 1).reshape(BS, L, MSL, V)

    viable = (np.arange(L)[:, None] + np.arange(MSL)[None, :])[None] \
        < lengths[:, None, None]
    value = np.where(viable[..., None], value, np.float32(BIG))

    best_value = value.min(axis=-1)
    matched_vocab = value.argmin(axis=-1)
    lens = vl[matched_vocab].astype(np.float32)
    matched = best_value < np.float32(MATCH_THRESH)
    score = lens * matched.astype(np.float32) * (np.float32(1.0) - best_value)

    sf = score.reshape(BS, -1)
    best_scores = sf.max(axis=-1)
    best_inds = sf.argmax(axis=-1).astype(np.int32)
    best_starts = best_inds // MSL
    best_ends = best_inds % MSL + best_starts
    matched_any = matched.reshape(BS, -1).any(axis=-1)
    return (best_scores.astype(np.float32), best_starts.astype(np.int32),
            best_ends.astype(np.int32), matched_any)


# revision 4
# speedup vs baseline: 3.4073x; 1.2731x over previous
"""Bass/Trainium2 kernel for nn_ExtractModel (soft banded edit-distance vocab matcher).

Sharding: vocab axis V=1000 split 8 x 125 across NeuronCores (partition dim = vocab).

v3 design (v1 baseline: 123.5 us, v2: 46.2 us):
  - ext[b,s,i] = word_repr[b, min(s+i, L-1)] has only L=48 distinct columns per
    batch, so the cosine matmul runs over the 192 distinct (b,l) pairs plus 9
    replicated clamp columns (228 total) instead of all 1920 windowed columns.
    Each DP band cell (i,j) reads a SHIFTED slice dpad[:, j-1, :, (i-1)+s] of
    the matmul output - the window gather becomes an access pattern.
  - bf16 matmul operands (4x PE vs fp32; dot error ~1e-3 against a 0.3 margin
    on the 0.05 match threshold).
  - h-space DP: h[i,j] = f[i,j] - (i+j). The ins/del "+1" transitions become
    "+0" and every boundary constant becomes 0, so each band cell is only
    TENSOR_TENSOR min/add and TENSOR_SCALAR ops - these hit the DVE 16-bit
    fast path on HW (~210 ns for [125,192] fp16), unlike SCALAR_TENSOR_TENSOR
    (~350 ns, measured). The sub-path needs (h_sub + dij - 2); the "-2" is
    folded into the ACT bias: dpad = -1.5 - 0.5*dot.  Host adds (i+j) back.
  - Input DMAs issued from the (otherwise idle) GpSimd queue; PE runs two
    passes over the contraction halves so matmuls start after 2/3 of input.
  - 5 double-width PSUM banks hold all 10 j-groups: no bank-reuse WAR waits.
  - ACT warmup op hides the one-time ACT_TABLE_LOAD (~1.3 us) off the
    critical path.
  - fp16 output DMA'd in 12 chunks of 3 band cells, overlapped with the DP.

Raw Bass (no TileContext); all cross-engine syncs are standalone wait_ge
instructions (walrus rejects >1 attached wait):
  s_w/s_v0/s_v1 : word / vocab-kc0 / vocab-kc1 input DMAs done (GpSimd -> PE)
  s_pe          : per-j psum group finished (PE pass 2 -> ACT)
  s_act         : dpad j-slices in SBUF     (ACT -> DVE)
  s_dve         : band cells finished       (DVE -> SP stores, 3-cell chunks)
  s_out         : output DMAs done

The reference's second DP table (not_viable init, all-BIG) provably yields
values >= 99.9 > MATCH_THRESH everywhere, so non-viable positions always score
exactly +/-0.0 and never match; constant BIG gives identical final outputs.
Out-of-band DP predecessors (>= BIG) never win the min and are dropped.
Shapes hardcoded per the problem spec.
"""

import numpy as np
import ml_dtypes

import concourse.bass as bass
import concourse.mybir as mybir
from concourse.bass_utils import run_bass_kernel_spmd

MSL = 10
MTL = 10
BIG = 99.9
MATCH_THRESH = 0.05
BS, L, D, V = 4, 48, 256, 1000
NCORES = 8
VC = V // NCORES          # 125 vocab words per core
M = BS * L                # 192 (b,s) positions
LP = L + MSL - 1          # 57 padded l columns (48 real + 9 clamp copies)
KC = D // 128             # 2 contraction chunks
F32 = mybir.dt.float32
F16 = mybir.dt.float16
BF16 = mybir.dt.bfloat16

# band cells of the edit-distance DP, in dependency order
BAND = [(i, j) for i in range(1, MSL + 1)
        for j in range(max(i - 2, 1), min(i + 2, MTL + 1))]
BAND_IDX = {c: n for n, c in enumerate(BAND)}
NCELL = len(BAND)         # 36
CHUNK = 3                 # band cells per output DMA chunk

_prog_cache = {}


def _build_program():
    nc = bass.Bass()
    wordT = nc.dram_tensor("wordT", [128, KC, BS, LP], BF16, kind="ExternalInput")
    vocabT = nc.dram_tensor("vocabT", [128, KC, MTL, VC], BF16, kind="ExternalInput")
    fband = nc.dram_tensor("fband", [VC, NCELL, BS, L], F16, kind="ExternalOutput")

    import contextlib
    with contextlib.ExitStack() as ctx:
        ent = ctx.enter_context
        word_s = ent(nc.sbuf_tensor("word_s", [128, KC, BS, LP], BF16))
        voc_s = ent(nc.sbuf_tensor("voc_s", [128, KC, MTL, VC], BF16))
        dpad = ent(nc.sbuf_tensor("dpad", [VC, MTL, BS, LP], F16))
        fall = ent(nc.sbuf_tensor("fall", [VC, NCELL, BS, L], F16))
        tmpA = ent(nc.sbuf_tensor("tmpA", [VC, BS, L], F16))
        tmpB = ent(nc.sbuf_tensor("tmpB", [VC, BS, L], F16))
        warm = ent(nc.sbuf_tensor("warm", [VC, 2], F32))
        # 5 double-width psum banks hold all 10 j-groups (1824 B <= 2 KiB bank)
        ps = [ent(nc.psum_tensor(f"ps{b}", [VC, 2, BS, LP], F32)) for b in range(5)]
        s_w = ent(nc.semaphore("s_w"))
        s_v0 = ent(nc.semaphore("s_v0"))
        s_v1 = ent(nc.semaphore("s_v1"))
        s_pe = ent(nc.semaphore("s_pe"))
        s_act = ent(nc.semaphore("s_act"))
        s_dve = ent(nc.semaphore("s_dve"))
        s_out = ent(nc.semaphore("s_out"))

        with nc.Block() as block:

            @block.gpsimd
            def _(gpsimd):
                gpsimd.dma_start(word_s[:], wordT[:]).then_inc(s_w, 16)
                gpsimd.dma_start(voc_s[:, 0], vocabT[:, 0]).then_inc(s_v0, 16)
                gpsimd.dma_start(voc_s[:, 1], vocabT[:, 1]).then_inc(s_v1, 16)

            @block.sync
            def _(sync):
                nq = NCELL // CHUNK
                for q in range(nq):
                    sync.wait_ge(s_dve, CHUNK * (q + 1))
                    sync.dma_start(fband[:, q * CHUNK:(q + 1) * CHUNK],
                                   fall[:, q * CHUNK:(q + 1) * CHUNK]
                                   ).then_inc(s_out, 16)
                sync.wait_ge(s_out, 16 * nq)

            @block.tensor
            def _(tensor):
                tensor.wait_ge(s_w, 16)
                tensor.wait_ge(s_v0, 16)
                for j in range(MTL):
                    tensor.matmul(ps[j // 2][:, j % 2], voc_s[:, 0, j, :],
                                  word_s[:, 0], start=True, stop=False)
                tensor.wait_ge(s_v1, 16)
                for j in range(MTL):
                    mm = tensor.matmul(ps[j // 2][:, j % 2], voc_s[:, 1, j, :],
                                       word_s[:, 1], start=False, stop=True)
                    mm.then_inc(s_pe, 1)

            @block.scalar
            def _(scalar):
                # warmup: pull the one-time ACT table load off the critical path
                scalar.activation(warm[:], tmpA[:, 0, 0:2],
                                  mybir.ActivationFunctionType.Copy)
                for j in range(MTL):
                    scalar.wait_ge(s_pe, j + 1)
                    scalar.activation(
                        dpad[:, j], ps[j // 2][:, j % 2],
                        mybir.ActivationFunctionType.Copy, bias=-1.5, scale=-0.5,
                    ).then_inc(s_act, 1)

            @block.vector
            def _(vector):
                Alu = mybir.AluOpType
                fmap = {}

                def pred(i, j):
                    # h-space: boundary rows/cols are all 0; out-of-band is BIG
                    if (i, j) in fmap:
                        return fmap[(i, j)]
                    if i == 0 or j == 0:
                        return 0.0
                    return BIG

                waited = 0
                for (i, j) in BAND:
                    dij = dpad[:, j - 1, :, i - 1:i - 1 + L]  # = diff - 2
                    if j > waited:
                        vector.wait_ge(s_act, j)
                        waited = j

                    sub_p = pred(i - 1, j - 1)
                    tens = [p for p in (pred(i - 1, j), pred(i, j - 1))
                            if not isinstance(p, float)]
                    n = BAND_IDX[(i, j)]
                    fcell = fall[:, n]
                    fmap[(i, j)] = fcell
                    tmps = [tmpA[:], tmpB[:]]
                    nops = 1 + len(tens)
                    k = 0

                    def out_of(k):
                        return fcell if k == nops - 1 else tmps[k]

                    if isinstance(sub_p, float):
                        # sub pred is the 0 boundary: min(dij' + 0, 0) also
                        # covers the (always present) 0-const ins/del pred
                        ins = vector.tensor_scalar(
                            out_of(k), dij, 0.0, 0.0, Alu.add, Alu.min)
                    else:
                        ins = vector.tensor_tensor(
                            out_of(k), sub_p, dij, Alu.add)
                    acc = out_of(k)
                    k += 1
                    for t in tens:
                        ins = vector.tensor_tensor(out_of(k), acc, t, Alu.min)
                        acc = out_of(k)
                        k += 1
                    ins.then_inc(s_dve, 1)

    return nc


def kernel(word_repr, vocab_repr, lengths, vocab_length):
    word_repr = np.asarray(word_repr, dtype=np.float32)
    vocab_repr = np.asarray(vocab_repr, dtype=np.float32)
    lengths = np.asarray(lengths)
    vocab_length = np.asarray(vocab_length)

    # ----- host prep: cosine pre-normalization + clamp-column replication -----
    nx = np.sqrt((word_repr * word_repr).sum(-1, dtype=np.float32)) + np.float32(1e-8)
    wordn = word_repr / nx[..., None]                            # [bs,L,d]
    ny = np.sqrt((vocab_repr * vocab_repr).sum(-1, dtype=np.float32)) + np.float32(1e-8)
    vocn = vocab_repr / ny[..., None]                            # [V,MTL,d]

    # wpad[b, l', d]: l' 0..47 real, 48..56 copies of column 47 (window clamp)
    wpad = np.concatenate(
        [wordn, np.repeat(wordn[:, L - 1:L, :], MSL - 1, axis=1)], axis=1)
    # wordT[k, kc, b, l']  (d = kc*128 + k)
    wordT = np.ascontiguousarray(
        wpad.transpose(2, 0, 1).reshape(KC, 128, BS, LP)
        .transpose(1, 0, 2, 3)).astype(ml_dtypes.bfloat16)
    in_maps = []
    for c in range(NCORES):
        vs = vocn[c * VC:(c + 1) * VC]                           # [125,10,256]
        vT = np.ascontiguousarray(
            vs.transpose(2, 1, 0).reshape(KC, 128, MTL, VC)
            .transpose(1, 0, 2, 3)).astype(ml_dtypes.bfloat16)
        in_maps.append({"wordT": wordT, "vocabT": vT})

    # ----- device: matmul + banded DP on 8 cores -----
    global _last_in_maps
    _last_in_maps = in_maps
    if "nc" not in _prog_cache:
        _prog_cache["nc"] = _build_program()
    res = run_bass_kernel_spmd(_prog_cache["nc"], in_maps, list(range(NCORES)))
    fb = np.stack([res.results[c]["fband"].reshape(VC, NCELL, M)
                   .transpose(1, 0, 2) for c in range(NCORES)]
                  ).astype(np.float32)                           # [8,36,125,192]

    # ----- host finish: gather at vocab_length, min over V, score, argmax -----
    f_full = np.full((MSL + 1, MTL + 1, NCORES, VC, M), BIG, dtype=np.float32)
    for n, (i, j) in enumerate(BAND):
        f_full[i, j] = fb[:, n] + np.float32(i + j)   # undo h-space shift
    vl = vocab_length.astype(np.int64)                           # [1000] in 1..10
    v_core = np.arange(V) // VC
    v_loc = np.arange(V) % VC
    # val2[e, v, m] = f[e+1, vl[v], v]
    val2 = f_full[np.arange(1, MSL + 1)[:, None], vl[None, :],
                  v_core[None, :], v_loc[None, :], :]            # [10,1000,192]
    value = val2.transpose(2, 0, 1).reshape(BS, L, MSL, V)

    viable = (np.arange(L)[:, None] + np.arange(MSL)[None, :])[None] \
        < lengths[:, None, None]
    value = np.where(viable[..., None], value, np.float32(BIG))

    best_value = value.min(axis=-1)
    matched_vocab = value.argmin(axis=-1)
    lens = vl[matched_vocab].astype(np.float32)
    matched = best_value < np.float32(MATCH_THRESH)
    score = lens * matched.astype(np.float32) * (np.float32(1.0) - best_value)

    sf = score.reshape(BS, -1)
    best_scores = sf.max(axis=-1)
    best_inds = sf.argmax(axis=-1).astype(np.int32)
    best_starts = best_inds // MSL
    best_ends = best_inds % MSL + best_starts
    matched_any = matched.reshape(BS, -1).any(axis=-1)
    return (best_scores.astype(np.float32), best_starts.astype(np.int32),
            best_ends.astype(np.int32), matched_any)
